# revision 19
# baseline (speedup 1.0000x reference)
"""Trainium2 Bass kernel for nn_AUAttnProcessor (AU-token attention processor).

Sharding: 8 cores = (batch b, head-group hg). Core c handles batch c//2 and
heads [4*(c%2), 4*(c%2)+4) (Ch=320 of C=640 channels).  Wq/Wk/Wv/Wak/Wav are
column-sharded, Wo row-sharded; each core emits a partial [S, C] output and the
host reduces the two partials per batch and adds bias + residual.

Per-core pipeline (transposed flash-attention orientation, bf16 operands,
inputs pre-cast to bf16 on the host):
  qT/kT = (W.T @ hsT)            [80, S] per head, evacuated bf16
  vaug  = hs @ Wv  + ones col 80 [128, sc, h, 82] bf16
  scoresT[kc] = kT_chunk.T @ qT  -> PSUM f32 [128, 1024]
  expT = Exp(scoresT) (ACT)      -> SBUF bf16
  outT += vaug_chunk.T @ expT    [82, 1024] PSUM; row 80 = softmax denominator
  raw_m = cast(outT) bf16        [82, NH, S] (row 80 = denominators)
  denominators: cast-DMA rows -> dsum[4,S] f32 -> reciprocal_approx_fast
                -> cast-DMA -> rrows [1, 2, NH, S] bf16 (partition 0)
  bc = partition_broadcast(rrows row)  (GpSimd, SBUF bf16)
  osb = raw * bc  (DVE bf16 2x mode)   per head and branch
  Wo: accumulate 16 matmuls (main+au, 4 heads) per 128-query chunk -> [128, 640]
"""

import os
import sys

import numpy as np

for _p in ("/opt/trn_rl_repo",):
    if os.path.isdir(_p) and _p not in sys.path:
        sys.path.insert(0, _p)

import ml_dtypes

import concourse.bass as bass
import concourse.tile as tile
from concourse import bacc, mybir
from concourse.bass_utils import run_bass_kernel_spmd

# Problem dims
B, S, C, H, D = 4, 2048, 640, 8, 80
NH = 4            # heads per core
CH = NH * D       # 320 channels per core
KC = C // 128     # 5 contraction chunks
SC = S // 128     # 16 sequence chunks
NAU = 13          # 12 AU tokens + 1 null token
NAUP = 14         # padded to even
VA = 82           # vaug rows: 80 v + ones row (80) + zero pad (81)
QB = 1024         # q-block width
NQB = S // QB
SCALE = float(D) ** -0.5

F32 = mybir.dt.float32
BF16 = mybir.dt.bfloat16
EXP = mybir.ActivationFunctionType.Exp
BF16_NP = ml_dtypes.bfloat16


def _phase_p(nc, tc, pers_tiles, raw_au, dram):
    """Projections + AU attention. Closes its pools on exit (frees hsT_sb)."""
    qT, kT, vaug, wo_sb, aukT, auvaug = pers_tiles
    hsT, wq, wk, wv, wak, wav, wo, extT, extzT = dram
    with tc.tile_pool(name="projp", bufs=1) as projp, \
         tc.tile_pool(name="wts", bufs=2) as wpool, \
         tc.tile_pool(name="ppsum", bufs=4, space="PSUM") as pps:
        ext_sb = projp.tile([128, KC, NAUP], BF16, name="ext_sb")
        nc.sync.dma_start(
            out=ext_sb, in_=extT[:].rearrange("(c p) n -> p c n", p=128))
        extz_sb = projp.tile([128, KC, NAUP], BF16, name="extz_sb")
        nc.sync.dma_start(
            out=extz_sb, in_=extzT[:].rearrange("(c p) n -> p c n", p=128))
        hsT_sb = projp.tile([128, KC, S], BF16, name="hsT_sb")
        nc.sync.dma_start(
            out=hsT_sb, in_=hsT[:].rearrange("(c p) s -> p c s", p=128))

        # au_k projection first: tiny matmuls warm the PE while the hsT DMA
        # is in flight
        w_sb = wpool.tile([128, KC, CH], BF16, tag="w", name="wak_sb")
        nc.sync.dma_start(
            out=w_sb, in_=wak[:].rearrange("(c p) n -> p c n", p=128))
        for h in range(NH):
            ps = pps.tile([D, NAUP], F32, tag="pp", name="ps_auk")
            for c in range(KC):
                nc.tensor.matmul(
                    ps,
                    w_sb[:, c, h * D:(h + 1) * D],
                    ext_sb[:, c, :],
                    start=(c == 0), stop=(c == KC - 1),
                )
            nc.vector.tensor_copy(aukT[:, h, :], ps)

        # au_v projection (natural [14, 320], gamma pre-folded on host)
        w_sb = wpool.tile([128, KC, CH], BF16, tag="w", name="wav_sb")
        nc.sync.dma_start(
            out=w_sb, in_=wav[:].rearrange("(c p) n -> p c n", p=128))
        ps = pps.tile([NAUP, CH], F32, tag="pp", name="ps_auv")
        for c in range(KC):
            nc.tensor.matmul(
                ps,
                extz_sb[:, c, :],
                w_sb[:, c, :],
                start=(c == 0), stop=(c == KC - 1),
            )
        nc.vector.tensor_copy(
            auvaug[:, :, 0:80], ps.rearrange("p (h d) -> p h d", d=D))
        nc.sync.dma_start(
            out=wo_sb, in_=wo[:].rearrange("(h d) n -> d h n", d=D))

        # q and k projections, per head (transposed output, bf16).
        # wq is DMA'd before hsT so the queue order is wq -> hsT -> wk: the
        # q projection can start the moment hsT lands.
        for wdram, dstT, evac in ((wq, qT, "act"), (wk, kT, "dve")):
            w_sb = wpool.tile([128, KC, CH], BF16, tag="w", name="w_sb")
            nc.sync.dma_start(
                out=w_sb, in_=wdram[:].rearrange("(c p) n -> p c n", p=128))
            for h in range(NH):
                for nb in range(S // 512):
                    ps = pps.tile([D, 512], F32, tag="pp", name="ps_qk")
                    for c in range(KC):
                        nc.tensor.matmul(
                            ps,
                            w_sb[:, c, h * D:(h + 1) * D],
                            hsT_sb[:, c, nb * 512:(nb + 1) * 512],
                            start=(c == 0), stop=(c == KC - 1),
                        )
                    if evac == "act":
                        nc.scalar.copy(dstT[:, h, nb * 512:(nb + 1) * 512], ps)
                    else:
                        nc.vector.tensor_copy(
                            dstT[:, h, nb * 512:(nb + 1) * 512], ps)

        # v projection (natural layout, strided into vaug)
        w_sb = wpool.tile([128, KC, CH], BF16, tag="w", name="wv_sb")
        nc.sync.dma_start(
            out=w_sb, in_=wv[:].rearrange("(c p) n -> p c n", p=128))
        for sc in range(SC):
            ps = pps.tile([128, CH], F32, tag="pp", name="ps_v")
            for c in range(KC):
                nc.tensor.matmul(
                    ps,
                    hsT_sb[:, c, sc * 128:(sc + 1) * 128],
                    w_sb[:, c, :],
                    start=(c == 0), stop=(c == KC - 1),
                )
            nc.vector.tensor_copy(
                vaug[:, sc, :, 0:80], ps.rearrange("p (h d) -> p h d", d=D))


def build_nc():
    nc = bacc.Bacc()
    hsT = nc.dram_tensor("hsT", [C, S], BF16, kind="ExternalInput")
    wq = nc.dram_tensor("wq", [C, CH], BF16, kind="ExternalInput")
    wk = nc.dram_tensor("wk", [C, CH], BF16, kind="ExternalInput")
    wv = nc.dram_tensor("wv", [C, CH], BF16, kind="ExternalInput")
    wak = nc.dram_tensor("wak", [C, CH], BF16, kind="ExternalInput")
    wav = nc.dram_tensor("wav", [C, CH], BF16, kind="ExternalInput")
    wo = nc.dram_tensor("wo", [CH, C], BF16, kind="ExternalInput")
    extT = nc.dram_tensor("extT", [C, NAUP], BF16, kind="ExternalInput")
    extzT = nc.dram_tensor("extzT", [C, NAUP], BF16, kind="ExternalInput")
    outp = nc.dram_tensor("outp", [S, C], F32, kind="ExternalOutput")
    dram = (hsT, wq, wk, wv, wak, wav, wo, extT, extzT)

    with tile.TileContext(nc) as tc, \
         nc.allow_low_precision(reason="bf16 attention pipeline; approx recip"):
        with tc.tile_pool(name="pers", bufs=1) as pers:
            qT = pers.tile([D, NH, S], BF16, name="qT")
            kT = pers.tile([D, NH, S], BF16, name="kT")
            vaug = pers.tile([128, SC, NH, VA], BF16, name="vaug")
            wo_sb = pers.tile([D, NH, C], BF16, name="wo_sb")
            aukT = pers.tile([D, NH, NAUP], BF16, name="aukT")
            auvaug = pers.tile([NAUP, NH, VA], BF16, name="auvaug")
            ones80 = pers.tile([1, D], BF16, name="ones80")

            nc.vector.memset(ones80, 1.0)
            nc.vector.memset(vaug[:, :, :, 80:81], 1.0)
            nc.vector.memset(vaug[:, :, :, 81:82], 0.0)
            # auvaug: col 80 = 1 on rows 0:13 only (pad row 13 excluded from
            # the denominator), col 81 = 0
            nc.vector.memset(auvaug[:, :, 80:82], 0.0)
            nc.vector.memset(auvaug[0:NAU, :, 80:81], 1.0)

            with tc.tile_pool(name="pmid", bufs=1) as pmid:
                raw_au = pmid.tile([VA, NH, S], BF16, name="raw_au")

                _phase_p(nc, tc, (qT, kT, vaug, wo_sb, aukT, auvaug),
                         raw_au, dram)

                # Persistent tiles for phases B/C (allocated after the
                # projection pool closed, reusing hsT_sb's space)
                with tc.tile_pool(name="pers2", bufs=1) as pers2, \
                     tc.tile_pool(name="bcp", bufs=3) as bcp:
                    raw_m = pers2.tile([VA, NH, S], BF16, name="raw_m")
                    osb_m = pers2.tile([D, NH, S], BF16, name="osb_m")
                    osb_a = pers2.tile([D, NH, S], BF16, name="osb_a")
                    dsum_m = pers2.tile([4, S], F32, name="dsum_m")
                    dsum_a = pers2.tile([4, S], F32, name="dsum_a")
                    dsum_rm = pers2.tile([4, S], F32, name="dsum_rm")
                    dsum_ra = pers2.tile([4, S], F32, name="dsum_ra")
                    rrows = pers2.tile([1, 2, NH, S], BF16, name="rrows")

                    # ---------------- Phase B: main attention ----------------
                    with tc.tile_pool(name="spool", bufs=2, space="PSUM") as spool, \
                         tc.tile_pool(name="opool", bufs=2, space="PSUM") as opool, \
                         tc.tile_pool(name="expp", bufs=3) as expp, \
                         tc.tile_pool(name="outp_sb", bufs=3) as outsb_pool:
                        # AU branch attention first, inside the same pools
                        # (no PSUM pool transition before the main loop)
                        for h in range(NH):
                            for hf in range(2):
                                f0 = hf * 1024
                                aus = spool.tile(
                                    [128, QB], F32, tag="sc", name="aus")
                                for nb in range(2):
                                    nc.tensor.matmul(
                                        aus[0:NAUP, nb * 512:(nb + 1) * 512],
                                        aukT[:, h, :],
                                        qT[:, h, f0 + nb * 512:f0 + (nb + 1) * 512],
                                        start=True, stop=True,
                                    )
                                au_e = expp.tile(
                                    [128, QB], BF16, tag="ex", name="au_e")
                                nc.scalar.activation(
                                    out=au_e[0:NAUP, :], in_=aus[0:NAUP, :],
                                    func=EXP)
                                auo = opool.tile(
                                    [VA, QB], F32, tag="ot", name="auo")
                                for nb in range(2):
                                    nc.tensor.matmul(
                                        auo[:, nb * 512:(nb + 1) * 512],
                                        auvaug[:, h, :],
                                        au_e[0:NAUP, nb * 512:(nb + 1) * 512],
                                        start=True, stop=True,
                                    )
                                nc.vector.tensor_copy(
                                    raw_au[:, h, f0:f0 + 1024], auo)

                        # au denominator pipeline + merge (overlaps early
                        # main attention)
                        nc.gpsimd.dma_start(out=dsum_a, in_=raw_au[80:81, :, :])
                        nc.vector.reciprocal_approx_fast(
                            out=dsum_ra, in_=dsum_a)
                        nc.gpsimd.dma_start(
                            out=rrows[0:1, 1, :, :], in_=dsum_ra)
                        for h in range(NH):
                            bc = bcp.tile([D, S], BF16, tag="bca", name="bc_a")
                            nc.gpsimd.partition_broadcast(
                                bc, rrows[0:1, 1, h, :])
                            nc.vector.tensor_mul(
                                osb_a[:, h, :], raw_au[0:80, h, :], bc)

                        def emit_wo(sj, outsb_pool, opool):
                            # Wo projection for query chunk sj; wo_ps shares
                            # the "ot" PSUM ring with outT (same pool/banks)
                            s0 = sj * 128
                            wo_ps = opool.tile(
                                [128, C], F32, tag="ot", name="wo_ps")
                            for n0, n1 in ((0, 512), (512, 640)):
                                k = 0
                                for osb in (osb_a, osb_m):
                                    for h in range(NH):
                                        nc.tensor.matmul(
                                            wo_ps[:, n0:n1],
                                            osb[:, h, s0:s0 + 128],
                                            wo_sb[:, h, n0:n1],
                                            start=(k == 0), stop=(k == 7),
                                        )
                                        k += 1
                            o_sb = outsb_pool.tile(
                                [128, C], F32, tag="ob", name="o_sb")
                            nc.scalar.copy(o_sb, wo_ps)
                            nc.sync.dma_start(
                                out=outp[s0:s0 + 128, :], in_=o_sb)

                        for qb in range(NQB):
                            q0 = qb * QB
                            for h in range(NH):
                                outT = opool.tile(
                                    [VA, QB], F32, tag="ot", name="outT")
                                for kc in range(SC):
                                    sco = spool.tile(
                                        [128, QB], F32, tag="sc", name="sco")
                                    for nn in range(QB // 512):
                                        nc.tensor.matmul(
                                            sco[:, nn * 512:(nn + 1) * 512],
                                            kT[:, h, kc * 128:(kc + 1) * 128],
                                            qT[:, h,
                                               q0 + nn * 512:q0 + (nn + 1) * 512],
                                            start=True, stop=True,
                                        )
                                    ex = expp.tile(
                                        [128, QB], BF16, tag="ex", name="ex")
                                    nc.scalar.activation(
                                        out=ex, in_=sco, func=EXP)
                                    for nn in range(QB // 512):
                                        nc.tensor.matmul(
                                            outT[:, nn * 512:(nn + 1) * 512],
                                            vaug[:, kc, h, :],
                                            ex[:, nn * 512:(nn + 1) * 512],
                                            start=(kc == 0), stop=(kc == SC - 1),
                                        )
                                nc.vector.tensor_copy(
                                    raw_m[:, h, q0:q0 + QB], outT)

                            # denominator pipeline + merge for this q-block
                            # (overlaps the next q-block's attention)
                            nc.gpsimd.dma_start(
                                out=dsum_m[:, q0:q0 + QB],
                                in_=raw_m[80:81, :, q0:q0 + QB],
                            )
                            nc.vector.reciprocal_approx_fast(
                                out=dsum_rm[:, q0:q0 + QB],
                                in_=dsum_m[:, q0:q0 + QB])
                            nc.gpsimd.dma_start(
                                out=rrows[0:1, 0, :, q0:q0 + QB],
                                in_=dsum_rm[:, q0:q0 + QB],
                            )
                            for h in range(NH):
                                bc = bcp.tile(
                                    [D, QB], BF16, tag="bcm", name="bc_m")
                                nc.gpsimd.partition_broadcast(
                                    bc, rrows[0:1, 0, h, q0:q0 + QB])
                                nc.vector.tensor_mul(
                                    osb_m[:, h, q0:q0 + QB],
                                    raw_m[0:80, h, q0:q0 + QB],
                                    bc,
                                )

                        # tail: qb0's chunks run while qb1's merge chain
                        # completes, then qb1's chunks
                        for sj in range(S // 128):
                            emit_wo(sj, outsb_pool, opool)
    nc.compile()
    return nc


_NC_CACHE = {}
LAST_EXEC_NS = None
LAST_RESULT = None


def _get_nc():
    if "nc" not in _NC_CACHE:
        _NC_CACHE["nc"] = build_nc()
    return _NC_CACHE["nc"]


def make_in_maps(inputs):
    hs = np.asarray(inputs["hidden_states"], np.float32)
    au = np.asarray(inputs["au_embedding"], np.float32)
    Wq = np.asarray(inputs["Wq"], np.float32)
    Wk = np.asarray(inputs["Wk"], np.float32)
    Wv = np.asarray(inputs["Wv"], np.float32)
    Wak = np.asarray(inputs["Wak"], np.float32)
    Wav = np.asarray(inputs["Wav"], np.float32)
    null_token = np.asarray(inputs["null_token"], np.float32).reshape(1, C)
    gamma = np.asarray(inputs["gamma"], np.float32)
    Wo = np.asarray(inputs["Wo"], np.float32)

    Wq_s = Wq * SCALE
    Wav_g = Wav * gamma[None, :]

    def b16(x):
        return np.ascontiguousarray(x.astype(BF16_NP))

    in_maps = []
    for c in range(8):
        b, hg = divmod(c, 2)
        sl = slice(hg * CH, (hg + 1) * CH)
        ext = np.concatenate(
            [au[b], null_token, np.zeros((1, C), np.float32)], axis=0
        )  # [14, C]; row 13 is even-size padding
        extz = ext.copy()
        extz[NAU - 1] = 0.0
        in_maps.append({
            "hsT": b16(hs[b].T),
            "wq": b16(Wq_s[:, sl]),
            "wk": b16(Wk[:, sl]),
            "wv": b16(Wv[:, sl]),
            "wak": b16(Wak[:, sl]),
            "wav": b16(Wav_g[:, sl]),
            "wo": b16(Wo[sl, :]),
            "extT": b16(ext.T),
            "extzT": b16(extz.T),
        })
    return in_maps


def kernel(**inputs):
    global LAST_EXEC_NS, LAST_RESULT
    hs = np.asarray(inputs["hidden_states"], np.float32)
    bo = np.asarray(inputs["bo"], np.float32)
    in_maps = make_in_maps(inputs)
    nc = _get_nc()
    trace = os.environ.get("KERNEL_TRACE", "0") == "1"
    res = run_bass_kernel_spmd(nc, in_maps, list(range(8)), trace=trace)
    LAST_EXEC_NS = res.exec_time_ns
    LAST_RESULT = res
    out = np.empty((B, S, C), np.float32)
    for b in range(B):
        out[b] = res.results[2 * b]["outp"] + res.results[2 * b + 1]["outp"]
        out[b] += bo[None, :]
        out[b] += hs[b]
    return out


# revision 20
# speedup vs baseline: 1.0256x; 1.0256x over previous
"""Trainium2 Bass kernel for nn_AUAttnProcessor (AU-token attention processor).

Sharding: 8 cores = (batch b, head-group hg). Core c handles batch c//2 and
heads [4*(c%2), 4*(c%2)+4) (Ch=320 of C=640 channels).  Wq/Wk/Wv/Wak/Wav are
column-sharded, Wo row-sharded; each core emits a partial [S, C] output and the
host reduces the two partials per batch and adds bias + residual.

Per-core pipeline (transposed flash-attention orientation, bf16 operands,
inputs pre-cast to bf16 on the host):
  qT/kT = (W.T @ hsT)            [80, S] per head, evacuated bf16
  vaug  = hs @ Wv  + ones col 80 [128, sc, h, 82] bf16
  scoresT[kc] = kT_chunk.T @ qT  -> PSUM f32 [128, 1024]
  expT = Exp(scoresT) (ACT)      -> SBUF bf16
  outT += vaug_chunk.T @ expT    [82, 1024] PSUM; row 80 = softmax denominator
  raw_m = cast(outT) bf16        [82, NH, S] (row 80 = denominators)
  denominators: cast-DMA rows -> dsum[4,S] f32 -> reciprocal_approx_fast
                -> cast-DMA -> rrows [1, 2, NH, S] bf16 (partition 0)
  bc = partition_broadcast(rrows row)  (GpSimd, SBUF bf16)
  osb = raw * bc  (DVE bf16 2x mode)   per head and branch
  Wo: accumulate 16 matmuls (main+au, 4 heads) per 128-query chunk -> [128, 640]
"""

import os
import sys

import numpy as np

for _p in ("/opt/trn_rl_repo",):
    if os.path.isdir(_p) and _p not in sys.path:
        sys.path.insert(0, _p)

import ml_dtypes

import concourse.bass as bass
import concourse.tile as tile
from concourse import bacc, mybir
from concourse.bass_utils import run_bass_kernel_spmd

# Problem dims
B, S, C, H, D = 4, 2048, 640, 8, 80
NH = 4            # heads per core
CH = NH * D       # 320 channels per core
KC = C // 128     # 5 contraction chunks
SC = S // 128     # 16 sequence chunks
NAU = 13          # 12 AU tokens + 1 null token
NAUP = 14         # padded to even
VA = 82           # vaug rows: 80 v + ones row (80) + zero pad (81)
QB = 1024         # q-block width
NQB = S // QB
SCALE = float(D) ** -0.5

F32 = mybir.dt.float32
BF16 = mybir.dt.bfloat16
EXP = mybir.ActivationFunctionType.Exp
BF16_NP = ml_dtypes.bfloat16


def _phase_p(nc, tc, pers_tiles, raw_au, dram):
    """Projections + AU attention. Closes its pools on exit (frees hsT_sb)."""
    qT, kT, vaug, wo_sb, aukT, auvaug = pers_tiles
    hsT, wq, wk, wv, wak, wav, wo, extT, extzT = dram
    with tc.tile_pool(name="projp", bufs=1) as projp, \
         tc.tile_pool(name="wts", bufs=2) as wpool, \
         tc.tile_pool(name="ppsum", bufs=4, space="PSUM") as pps:
        ext_sb = projp.tile([128, KC, NAUP], BF16, name="ext_sb")
        nc.sync.dma_start(
            out=ext_sb, in_=extT[:].rearrange("(c p) n -> p c n", p=128))
        extz_sb = projp.tile([128, KC, NAUP], BF16, name="extz_sb")
        nc.sync.dma_start(
            out=extz_sb, in_=extzT[:].rearrange("(c p) n -> p c n", p=128))
        hsT_sb = projp.tile([128, KC, S], BF16, name="hsT_sb")
        nc.sync.dma_start(
            out=hsT_sb, in_=hsT[:].rearrange("(c p) s -> p c s", p=128))

        # au_k projection first: tiny matmuls warm the PE while the hsT DMA
        # is in flight
        w_sb = wpool.tile([128, KC, CH], BF16, tag="w", name="wak_sb")
        nc.sync.dma_start(
            out=w_sb, in_=wak[:].rearrange("(c p) n -> p c n", p=128))
        for h in range(NH):
            ps = pps.tile([D, NAUP], F32, tag="pp", name="ps_auk")
            for c in range(KC):
                nc.tensor.matmul(
                    ps,
                    w_sb[:, c, h * D:(h + 1) * D],
                    ext_sb[:, c, :],
                    start=(c == 0), stop=(c == KC - 1),
                )
            nc.vector.tensor_copy(aukT[:, h, :], ps)

        # au_v projection (natural [14, 320], gamma pre-folded on host)
        w_sb = wpool.tile([128, KC, CH], BF16, tag="w", name="wav_sb")
        nc.sync.dma_start(
            out=w_sb, in_=wav[:].rearrange("(c p) n -> p c n", p=128))
        ps = pps.tile([NAUP, CH], F32, tag="pp", name="ps_auv")
        for c in range(KC):
            nc.tensor.matmul(
                ps,
                extz_sb[:, c, :],
                w_sb[:, c, :],
                start=(c == 0), stop=(c == KC - 1),
            )
        nc.vector.tensor_copy(
            auvaug[:, :, 0:80], ps.rearrange("p (h d) -> p h d", d=D))
        nc.sync.dma_start(
            out=wo_sb, in_=wo[:].rearrange("(h d) n -> d h n", d=D))

        # q and k projections, per head (transposed output, bf16).
        # wq is DMA'd before hsT so the queue order is wq -> hsT -> wk: the
        # q projection can start the moment hsT lands.
        for wdram, dstT, evac in ((wq, qT, "act"), (wk, kT, "dve")):
            w_sb = wpool.tile([128, KC, CH], BF16, tag="w", name="w_sb")
            nc.sync.dma_start(
                out=w_sb, in_=wdram[:].rearrange("(c p) n -> p c n", p=128))
            for h in range(NH):
                for nb in range(S // 512):
                    ps = pps.tile([D, 512], F32, tag="pp", name="ps_qk")
                    for c in range(KC):
                        nc.tensor.matmul(
                            ps,
                            w_sb[:, c, h * D:(h + 1) * D],
                            hsT_sb[:, c, nb * 512:(nb + 1) * 512],
                            start=(c == 0), stop=(c == KC - 1),
                        )
                    if evac == "act":
                        nc.scalar.copy(dstT[:, h, nb * 512:(nb + 1) * 512], ps)
                    else:
                        nc.vector.tensor_copy(
                            dstT[:, h, nb * 512:(nb + 1) * 512], ps)

        # v projection (natural layout, strided into vaug)
        w_sb = wpool.tile([128, KC, CH], BF16, tag="w", name="wv_sb")
        nc.sync.dma_start(
            out=w_sb, in_=wv[:].rearrange("(c p) n -> p c n", p=128))
        for sc in range(SC):
            ps = pps.tile([128, CH], F32, tag="pp", name="ps_v")
            for c in range(KC):
                nc.tensor.matmul(
                    ps,
                    hsT_sb[:, c, sc * 128:(sc + 1) * 128],
                    w_sb[:, c, :],
                    start=(c == 0), stop=(c == KC - 1),
                )
            nc.vector.tensor_copy(
                vaug[:, sc, :, 0:80], ps.rearrange("p (h d) -> p h d", d=D))


def build_nc():
    nc = bacc.Bacc()
    hsT = nc.dram_tensor("hsT", [C, S], BF16, kind="ExternalInput")
    wq = nc.dram_tensor("wq", [C, CH], BF16, kind="ExternalInput")
    wk = nc.dram_tensor("wk", [C, CH], BF16, kind="ExternalInput")
    wv = nc.dram_tensor("wv", [C, CH], BF16, kind="ExternalInput")
    wak = nc.dram_tensor("wak", [C, CH], BF16, kind="ExternalInput")
    wav = nc.dram_tensor("wav", [C, CH], BF16, kind="ExternalInput")
    wo = nc.dram_tensor("wo", [CH, C], BF16, kind="ExternalInput")
    extT = nc.dram_tensor("extT", [C, NAUP], BF16, kind="ExternalInput")
    extzT = nc.dram_tensor("extzT", [C, NAUP], BF16, kind="ExternalInput")
    outp = nc.dram_tensor("outp", [S, C], F32, kind="ExternalOutput")
    dram = (hsT, wq, wk, wv, wak, wav, wo, extT, extzT)

    with tile.TileContext(nc) as tc, \
         nc.allow_low_precision(reason="bf16 attention pipeline; approx recip"):
        with tc.tile_pool(name="pers", bufs=1) as pers:
            qT = pers.tile([D, NH, S], BF16, name="qT")
            kT = pers.tile([D, NH, S], BF16, name="kT")
            vaug = pers.tile([128, SC, NH, VA], BF16, name="vaug")
            wo_sb = pers.tile([D, NH, C], BF16, name="wo_sb")
            aukT = pers.tile([D, NH, NAUP], BF16, name="aukT")
            auvaug = pers.tile([NAUP, NH, VA], BF16, name="auvaug")
            ones80 = pers.tile([1, D], BF16, name="ones80")

            nc.vector.memset(ones80, 1.0)
            nc.vector.memset(vaug[:, :, :, 80:81], 1.0)
            nc.vector.memset(vaug[:, :, :, 81:82], 0.0)
            # auvaug: col 80 = 1 on rows 0:13 only (pad row 13 excluded from
            # the denominator), col 81 = 0
            nc.vector.memset(auvaug[:, :, 80:82], 0.0)
            nc.vector.memset(auvaug[0:NAU, :, 80:81], 1.0)

            with tc.tile_pool(name="pmid", bufs=1) as pmid:
                raw_au = pmid.tile([VA, NH, S], BF16, name="raw_au")

                _phase_p(nc, tc, (qT, kT, vaug, wo_sb, aukT, auvaug),
                         raw_au, dram)

                # Persistent tiles for phases B/C (allocated after the
                # projection pool closed, reusing hsT_sb's space)
                with tc.tile_pool(name="pers2", bufs=1) as pers2, \
                     tc.tile_pool(name="bcp", bufs=3) as bcp:
                    raw_m = pers2.tile([VA, NH, S], BF16, name="raw_m")
                    osb_m = pers2.tile([D, NH, S], BF16, name="osb_m")
                    osb_a = pers2.tile([D, NH, S], BF16, name="osb_a")
                    dsum_m = pers2.tile([4, S], F32, name="dsum_m")
                    dsum_a = pers2.tile([4, S], F32, name="dsum_a")
                    dsum_rm = pers2.tile([4, S], F32, name="dsum_rm")
                    dsum_ra = pers2.tile([4, S], F32, name="dsum_ra")
                    rrows = pers2.tile([1, 2, NH, S], BF16, name="rrows")

                    # ---------------- Phase B: main attention ----------------
                    with tc.tile_pool(name="spool", bufs=2, space="PSUM") as spool, \
                         tc.tile_pool(name="opool", bufs=2, space="PSUM") as opool, \
                         tc.tile_pool(name="expp", bufs=3) as expp, \
                         tc.tile_pool(name="outp_sb", bufs=3) as outsb_pool:
                        # AU branch attention first, inside the same pools
                        # (no PSUM pool transition before the main loop)
                        for h in range(NH):
                            for hf in range(2):
                                f0 = hf * 1024
                                aus = spool.tile(
                                    [128, QB], F32, tag="sc", name="aus")
                                for nb in range(2):
                                    nc.tensor.matmul(
                                        aus[0:NAUP, nb * 512:(nb + 1) * 512],
                                        aukT[:, h, :],
                                        qT[:, h, f0 + nb * 512:f0 + (nb + 1) * 512],
                                        start=True, stop=True,
                                    )
                                au_e = expp.tile(
                                    [128, QB], BF16, tag="ex", name="au_e")
                                nc.scalar.activation(
                                    out=au_e[0:NAUP, :], in_=aus[0:NAUP, :],
                                    func=EXP)
                                auo = opool.tile(
                                    [VA, QB], F32, tag="ot", name="auo")
                                for nb in range(2):
                                    nc.tensor.matmul(
                                        auo[:, nb * 512:(nb + 1) * 512],
                                        auvaug[:, h, :],
                                        au_e[0:NAUP, nb * 512:(nb + 1) * 512],
                                        start=True, stop=True,
                                    )
                                nc.vector.tensor_copy(
                                    raw_au[:, h, f0:f0 + 1024], auo)

                        # au denominator pipeline + merge (overlaps early
                        # main attention)
                        nc.gpsimd.dma_start(out=dsum_a, in_=raw_au[80:81, :, :])
                        nc.vector.reciprocal_approx_fast(
                            out=dsum_ra, in_=dsum_a)
                        nc.gpsimd.dma_start(
                            out=rrows[0:1, 1, :, :], in_=dsum_ra)
                        for h in range(NH):
                            bc = bcp.tile([D, S], BF16, tag="bca", name="bc_a")
                            nc.gpsimd.partition_broadcast(
                                bc, rrows[0:1, 1, h, :])
                            nc.vector.tensor_mul(
                                osb_a[:, h, :], raw_au[0:80, h, :], bc)

                        def emit_wo(sj, outsb_pool, opool, evac="act"):
                            # Wo projection for query chunk sj; wo_ps shares
                            # the "ot" PSUM ring with outT (same pool/banks)
                            s0 = sj * 128
                            wo_ps = opool.tile(
                                [128, C], F32, tag="ot", name="wo_ps")
                            for n0, n1 in ((0, 512), (512, 640)):
                                k = 0
                                for osb in (osb_a, osb_m):
                                    for h in range(NH):
                                        nc.tensor.matmul(
                                            wo_ps[:, n0:n1],
                                            osb[:, h, s0:s0 + 128],
                                            wo_sb[:, h, n0:n1],
                                            start=(k == 0), stop=(k == 7),
                                        )
                                        k += 1
                            o_sb = outsb_pool.tile(
                                [128, C], F32, tag="ob", name="o_sb")
                            if evac == "act":
                                nc.scalar.copy(o_sb, wo_ps)
                            else:
                                nc.vector.tensor_copy(o_sb, wo_ps)
                            nc.sync.dma_start(
                                out=outp[s0:s0 + 128, :], in_=o_sb)

                        for qb in range(NQB):
                            q0 = qb * QB
                            for h in range(NH):
                                outT = opool.tile(
                                    [VA, QB], F32, tag="ot", name="outT")
                                for kc in range(SC):
                                    sco = spool.tile(
                                        [128, QB], F32, tag="sc", name="sco")
                                    for nn in range(QB // 512):
                                        nc.tensor.matmul(
                                            sco[:, nn * 512:(nn + 1) * 512],
                                            kT[:, h, kc * 128:(kc + 1) * 128],
                                            qT[:, h,
                                               q0 + nn * 512:q0 + (nn + 1) * 512],
                                            start=True, stop=True,
                                        )
                                    ex = expp.tile(
                                        [128, QB], BF16, tag="ex", name="ex")
                                    nc.scalar.activation(
                                        out=ex, in_=sco, func=EXP)
                                    for nn in range(QB // 512):
                                        nc.tensor.matmul(
                                            outT[:, nn * 512:(nn + 1) * 512],
                                            vaug[:, kc, h, :],
                                            ex[:, nn * 512:(nn + 1) * 512],
                                            start=(kc == 0), stop=(kc == SC - 1),
                                        )
                                nc.vector.tensor_copy(
                                    raw_m[:, h, q0:q0 + QB], outT)
                                if qb == 1 and h >= 1:
                                    # interleave part of qb0's Wo into qb1's
                                    # attention (fills ACT-paced PE slack,
                                    # keeps the PE warm); sj 3..7 are held
                                    # back to feed the PE during the final
                                    # denominator/merge chain
                                    emit_wo(h - 1, outsb_pool, opool,
                                            evac="dve")

                            # denominator pipeline + merge for this q-block
                            # (overlaps the next q-block's attention)
                            nc.gpsimd.dma_start(
                                out=dsum_m[:, q0:q0 + QB],
                                in_=raw_m[80:81, :, q0:q0 + QB],
                            )
                            nc.vector.reciprocal_approx_fast(
                                out=dsum_rm[:, q0:q0 + QB],
                                in_=dsum_m[:, q0:q0 + QB])
                            nc.gpsimd.dma_start(
                                out=rrows[0:1, 0, :, q0:q0 + QB],
                                in_=dsum_rm[:, q0:q0 + QB],
                            )
                            for h in range(NH):
                                bc = bcp.tile(
                                    [D, QB], BF16, tag="bcm", name="bc_m")
                                nc.gpsimd.partition_broadcast(
                                    bc, rrows[0:1, 0, h, q0:q0 + QB])
                                nc.vector.tensor_mul(
                                    osb_m[:, h, q0:q0 + QB],
                                    raw_m[0:80, h, q0:q0 + QB],
                                    bc,
                                )

                        # tail: reserved qb0 chunks run while qb1's merge
                        # chain completes, then qb1's chunks; evac on the
                        # now-idle Scalar engine (DVE runs the merge muls)
                        for sj in range(3, S // 128):
                            emit_wo(sj, outsb_pool, opool, evac="act")
    nc.compile()
    return nc


_NC_CACHE = {}
LAST_EXEC_NS = None
LAST_RESULT = None


def _get_nc():
    if "nc" not in _NC_CACHE:
        _NC_CACHE["nc"] = build_nc()
    return _NC_CACHE["nc"]


def make_in_maps(inputs):
    hs = np.asarray(inputs["hidden_states"], np.float32)
    au = np.asarray(inputs["au_embedding"], np.float32)
    Wq = np.asarray(inputs["Wq"], np.float32)
    Wk = np.asarray(inputs["Wk"], np.float32)
    Wv = np.asarray(inputs["Wv"], np.float32)
    Wak = np.asarray(inputs["Wak"], np.float32)
    Wav = np.asarray(inputs["Wav"], np.float32)
    null_token = np.asarray(inputs["null_token"], np.float32).reshape(1, C)
    gamma = np.asarray(inputs["gamma"], np.float32)
    Wo = np.asarray(inputs["Wo"], np.float32)

    Wq_s = Wq * SCALE
    Wav_g = Wav * gamma[None, :]

    def b16(x):
        return np.ascontiguousarray(x.astype(BF16_NP))

    in_maps = []
    for c in range(8):
        b, hg = divmod(c, 2)
        sl = slice(hg * CH, (hg + 1) * CH)
        ext = np.concatenate(
            [au[b], null_token, np.zeros((1, C), np.float32)], axis=0
        )  # [14, C]; row 13 is even-size padding
        extz = ext.copy()
        extz[NAU - 1] = 0.0
        in_maps.append({
            "hsT": b16(hs[b].T),
            "wq": b16(Wq_s[:, sl]),
            "wk": b16(Wk[:, sl]),
            "wv": b16(Wv[:, sl]),
            "wak": b16(Wak[:, sl]),
            "wav": b16(Wav_g[:, sl]),
            "wo": b16(Wo[sl, :]),
            "extT": b16(ext.T),
            "extzT": b16(extz.T),
        })
    return in_maps


def kernel(**inputs):
    global LAST_EXEC_NS, LAST_RESULT
    hs = np.asarray(inputs["hidden_states"], np.float32)
    bo = np.asarray(inputs["bo"], np.float32)
    in_maps = make_in_maps(inputs)
    nc = _get_nc()
    trace = os.environ.get("KERNEL_TRACE", "0") == "1"
    res = run_bass_kernel_spmd(nc, in_maps, list(range(8)), trace=trace)
    LAST_EXEC_NS = res.exec_time_ns
    LAST_RESULT = res
    out = np.empty((B, S, C), np.float32)
    for b in range(B):
        out[b] = res.results[2 * b]["outp"] + res.results[2 * b + 1]["outp"]
        out[b] += bo[None, :]
        out[b] += hs[b]
    return out


# revision 21
# speedup vs baseline: 1.0950x; 1.0677x over previous
"""Trainium2 Bass kernel for nn_AUAttnProcessor (AU-token attention processor).

Sharding: 8 cores = (batch b, head-group hg). Core c handles batch c//2 and
heads [4*(c%2), 4*(c%2)+4) (Ch=320 of C=640 channels).  Wq/Wk/Wv/Wak/Wav are
column-sharded, Wo row-sharded; each core emits a partial [S, C] output and the
host reduces the two partials per batch and adds bias + residual.

Per-core pipeline (transposed flash-attention orientation, bf16 operands,
inputs pre-cast to bf16 on the host):
  qT/kT = (W.T @ hsT)            [80, S] per head, evacuated bf16
  vaug  = hs @ Wv  + ones col 80 [128, sc, h, 82] bf16
  scoresT[kc] = kT_chunk.T @ qT  -> PSUM f32 [128, 1024]
  expT = Exp(scoresT) (ACT)      -> SBUF bf16
  outT += vaug_chunk.T @ expT    [82, 1024] PSUM; row 80 = softmax denominator
  raw_m = cast(outT) bf16        [82, NH, S] (row 80 = denominators)
  denominators: cast-DMA rows -> dsum[4,S] f32 -> reciprocal_approx_fast
                -> cast-DMA -> rrows [1, 2, NH, S] bf16 (partition 0)
  bc = partition_broadcast(rrows row)  (GpSimd, SBUF bf16)
  osb = raw * bc  (DVE bf16 2x mode)   per head and branch
  Wo: accumulate 16 matmuls (main+au, 4 heads) per 128-query chunk -> [128, 640]
"""

import os
import sys

import numpy as np

for _p in ("/opt/trn_rl_repo",):
    if os.path.isdir(_p) and _p not in sys.path:
        sys.path.insert(0, _p)

import ml_dtypes

import concourse.bass as bass
import concourse.tile as tile
from concourse import bacc, mybir
from concourse.bass_utils import run_bass_kernel_spmd

# Problem dims
B, S, C, H, D = 4, 2048, 640, 8, 80
NH = 4            # heads per core
CH = NH * D       # 320 channels per core
KC = C // 128     # 5 contraction chunks
SC = S // 128     # 16 sequence chunks
NAU = 13          # 12 AU tokens + 1 null token
NAUP = 14         # padded to even
VA = 82           # vaug rows: 80 v + ones row (80) + zero pad (81)
QB = 1024         # q-block width
NQB = S // QB
SCALE = float(D) ** -0.5

F32 = mybir.dt.float32
BF16 = mybir.dt.bfloat16
EXP = mybir.ActivationFunctionType.Exp
BF16_NP = ml_dtypes.bfloat16


def _phase_p(nc, tc, pers_tiles, raw_au, dram):
    """Projections + AU attention. Closes its pools on exit (frees hsT_sb)."""
    qT, kT, vaug, wo_sb, aukT, auvaug = pers_tiles
    hsT, wq, wk, wv, wak, wav, wo, extT, extzT = dram
    with tc.tile_pool(name="projp", bufs=1) as projp, \
         tc.tile_pool(name="wts", bufs=2) as wpool, \
         tc.tile_pool(name="ppsum", bufs=4, space="PSUM") as pps:
        ext_sb = projp.tile([128, KC, NAUP], BF16, name="ext_sb")
        nc.sync.dma_start(
            out=ext_sb, in_=extT[:].rearrange("(c p) n -> p c n", p=128))
        extz_sb = projp.tile([128, KC, NAUP], BF16, name="extz_sb")
        nc.sync.dma_start(
            out=extz_sb, in_=extzT[:].rearrange("(c p) n -> p c n", p=128))
        hsT_sb = projp.tile([128, KC, S], BF16, name="hsT_sb")
        nc.sync.dma_start(
            out=hsT_sb, in_=hsT[:].rearrange("(c p) s -> p c s", p=128))

        # au_k projection first: tiny matmuls warm the PE while the hsT DMA
        # is in flight
        w_sb = wpool.tile([128, KC, CH], BF16, tag="w", name="wak_sb")
        nc.sync.dma_start(
            out=w_sb, in_=wak[:].rearrange("(c p) n -> p c n", p=128))
        for h in range(NH):
            ps = pps.tile([D, NAUP], F32, tag="pp", name="ps_auk")
            for c in range(KC):
                nc.tensor.matmul(
                    ps,
                    w_sb[:, c, h * D:(h + 1) * D],
                    ext_sb[:, c, :],
                    start=(c == 0), stop=(c == KC - 1),
                )
            nc.vector.tensor_copy(aukT[:, h, :], ps)

        # au_v projection (natural [14, 320], gamma pre-folded on host)
        w_sb = wpool.tile([128, KC, CH], BF16, tag="w", name="wav_sb")
        nc.sync.dma_start(
            out=w_sb, in_=wav[:].rearrange("(c p) n -> p c n", p=128))
        ps = pps.tile([NAUP, CH], F32, tag="pp", name="ps_auv")
        for c in range(KC):
            nc.tensor.matmul(
                ps,
                extz_sb[:, c, :],
                w_sb[:, c, :],
                start=(c == 0), stop=(c == KC - 1),
            )
        nc.vector.tensor_copy(
            auvaug[:, :, 0:80], ps.rearrange("p (h d) -> p h d", d=D))
        nc.sync.dma_start(
            out=wo_sb, in_=wo[:].rearrange("(h d) n -> d h n", d=D))

        # q and k projections, per head (transposed output, bf16).
        # wq is DMA'd before hsT so the queue order is wq -> hsT -> wk: the
        # q projection can start the moment hsT lands.
        for wdram, dstT, evac in ((wq, qT, "act"), (wk, kT, "dve")):
            w_sb = wpool.tile([128, KC, CH], BF16, tag="w", name="w_sb")
            nc.sync.dma_start(
                out=w_sb, in_=wdram[:].rearrange("(c p) n -> p c n", p=128))
            for h in range(NH):
                for nb in range(S // 512):
                    ps = pps.tile([D, 512], F32, tag="pp", name="ps_qk")
                    for c in range(KC):
                        nc.tensor.matmul(
                            ps,
                            w_sb[:, c, h * D:(h + 1) * D],
                            hsT_sb[:, c, nb * 512:(nb + 1) * 512],
                            start=(c == 0), stop=(c == KC - 1),
                        )
                    if evac == "act":
                        nc.scalar.copy(dstT[:, h, nb * 512:(nb + 1) * 512], ps)
                    else:
                        nc.vector.tensor_copy(
                            dstT[:, h, nb * 512:(nb + 1) * 512], ps)

        # v projection (natural layout, strided into vaug)
        w_sb = wpool.tile([128, KC, CH], BF16, tag="w", name="wv_sb")
        nc.sync.dma_start(
            out=w_sb, in_=wv[:].rearrange("(c p) n -> p c n", p=128))
        for sc in range(SC):
            ps = pps.tile([128, CH], F32, tag="pp", name="ps_v")
            for c in range(KC):
                nc.tensor.matmul(
                    ps,
                    hsT_sb[:, c, sc * 128:(sc + 1) * 128],
                    w_sb[:, c, :],
                    start=(c == 0), stop=(c == KC - 1),
                )
            nc.vector.tensor_copy(
                vaug[:, sc, :, 0:80], ps.rearrange("p (h d) -> p h d", d=D))


def build_nc():
    nc = bacc.Bacc()
    hsT = nc.dram_tensor("hsT", [C, S], BF16, kind="ExternalInput")
    wq = nc.dram_tensor("wq", [C, CH], BF16, kind="ExternalInput")
    wk = nc.dram_tensor("wk", [C, CH], BF16, kind="ExternalInput")
    wv = nc.dram_tensor("wv", [C, CH], BF16, kind="ExternalInput")
    wak = nc.dram_tensor("wak", [C, CH], BF16, kind="ExternalInput")
    wav = nc.dram_tensor("wav", [C, CH], BF16, kind="ExternalInput")
    wo = nc.dram_tensor("wo", [CH, C], BF16, kind="ExternalInput")
    extT = nc.dram_tensor("extT", [C, NAUP], BF16, kind="ExternalInput")
    extzT = nc.dram_tensor("extzT", [C, NAUP], BF16, kind="ExternalInput")
    outp = nc.dram_tensor("outp", [S, C], F32, kind="ExternalOutput")
    dram = (hsT, wq, wk, wv, wak, wav, wo, extT, extzT)

    with tile.TileContext(nc) as tc, \
         nc.allow_low_precision(reason="bf16 attention pipeline; approx recip"):
        with tc.tile_pool(name="pers", bufs=1) as pers:
            qT = pers.tile([D, NH, S], BF16, name="qT")
            kT = pers.tile([D, NH, S], BF16, name="kT")
            vaug = pers.tile([128, SC, NH, VA], BF16, name="vaug")
            wo_sb = pers.tile([D, NH, C], BF16, name="wo_sb")
            aukT = pers.tile([D, NH, NAUP], BF16, name="aukT")
            auvaug = pers.tile([NAUP, NH, VA], BF16, name="auvaug")
            ones80 = pers.tile([1, D], BF16, name="ones80")

            nc.vector.memset(ones80, 1.0)
            nc.vector.memset(vaug[:, :, :, 80:81], 1.0)
            nc.vector.memset(vaug[:, :, :, 81:82], 0.0)
            # auvaug: col 80 = 1 on rows 0:13 only (pad row 13 excluded from
            # the denominator), col 81 = 0
            nc.vector.memset(auvaug[:, :, 80:82], 0.0)
            nc.vector.memset(auvaug[0:NAU, :, 80:81], 1.0)

            with tc.tile_pool(name="pmid", bufs=1) as pmid:
                raw_au = pmid.tile([VA, NH, S], BF16, name="raw_au")

                _phase_p(nc, tc, (qT, kT, vaug, wo_sb, aukT, auvaug),
                         raw_au, dram)

                # Persistent tiles for phases B/C (allocated after the
                # projection pool closed, reusing hsT_sb's space)
                with tc.tile_pool(name="pers2", bufs=1) as pers2, \
                     tc.tile_pool(name="bcp", bufs=3) as bcp:
                    raw_m = pers2.tile([VA, NH, S], BF16, name="raw_m")
                    osb_m = pers2.tile([D, NH, S], BF16, name="osb_m")
                    osb_a = pers2.tile([D, NH, S], BF16, name="osb_a")
                    dsum4 = pers2.tile([4, NH, NQB, 256], F32, name="dsum4")
                    dsum4r = pers2.tile([4, NH, NQB, 256], F32, name="dsum4r")
                    dsum_a = pers2.tile([4, S], F32, name="dsum_a")
                    dsum_ra = pers2.tile([4, S], F32, name="dsum_ra")
                    rrows = pers2.tile([1, 2, NH, S], BF16, name="rrows")

                    # ---------------- Phase B: main attention ----------------
                    with tc.tile_pool(name="spool", bufs=2, space="PSUM") as spool, \
                         tc.tile_pool(name="opool", bufs=2, space="PSUM") as opool, \
                         tc.tile_pool(name="expp", bufs=3) as expp, \
                         tc.tile_pool(name="outp_sb", bufs=3) as outsb_pool:
                        # AU branch attention first, inside the same pools
                        # (no PSUM pool transition before the main loop)
                        for h in range(NH):
                            for hf in range(2):
                                f0 = hf * 1024
                                aus = spool.tile(
                                    [128, QB], F32, tag="sc", name="aus")
                                for nb in range(2):
                                    nc.tensor.matmul(
                                        aus[0:NAUP, nb * 512:(nb + 1) * 512],
                                        aukT[:, h, :],
                                        qT[:, h, f0 + nb * 512:f0 + (nb + 1) * 512],
                                        start=True, stop=True,
                                    )
                                au_e = expp.tile(
                                    [128, QB], BF16, tag="ex", name="au_e")
                                nc.scalar.activation(
                                    out=au_e[0:NAUP, :], in_=aus[0:NAUP, :],
                                    func=EXP)
                                auo = opool.tile(
                                    [VA, QB], F32, tag="ot", name="auo")
                                for nb in range(2):
                                    nc.tensor.matmul(
                                        auo[:, nb * 512:(nb + 1) * 512],
                                        auvaug[:, h, :],
                                        au_e[0:NAUP, nb * 512:(nb + 1) * 512],
                                        start=True, stop=True,
                                    )
                                nc.vector.tensor_copy(
                                    raw_au[:, h, f0:f0 + 1024], auo)

                        # au denominator pipeline + merge (overlaps early
                        # main attention)
                        nc.gpsimd.dma_start(out=dsum_a, in_=raw_au[80:81, :, :])
                        nc.vector.reciprocal_approx_fast(
                            out=dsum_ra, in_=dsum_a)
                        nc.gpsimd.dma_start(
                            out=rrows[0:1, 1, :, :], in_=dsum_ra)
                        for h in range(NH):
                            bc = bcp.tile([D, S], BF16, tag="bca", name="bc_a")
                            nc.gpsimd.partition_broadcast(
                                bc, rrows[0:1, 1, h, :])
                            nc.vector.tensor_mul(
                                osb_a[:, h, :], raw_au[0:80, h, :], bc)

                        def emit_wo(sj, outsb_pool, opool, evac="act"):
                            # Wo projection for query chunk sj; wo_ps shares
                            # the "ot" PSUM ring with outT (same pool/banks)
                            s0 = sj * 128
                            wo_ps = opool.tile(
                                [128, C], F32, tag="ot", name="wo_ps")
                            for n0, n1 in ((0, 512), (512, 640)):
                                k = 0
                                for osb in (osb_a, osb_m):
                                    for h in range(NH):
                                        nc.tensor.matmul(
                                            wo_ps[:, n0:n1],
                                            osb[:, h, s0:s0 + 128],
                                            wo_sb[:, h, n0:n1],
                                            start=(k == 0), stop=(k == 7),
                                        )
                                        k += 1
                            o_sb = outsb_pool.tile(
                                [128, C], F32, tag="ob", name="o_sb")
                            if evac == "act":
                                nc.scalar.copy(o_sb, wo_ps)
                            else:
                                nc.vector.tensor_copy(o_sb, wo_ps)
                            nc.sync.dma_start(
                                out=outp[s0:s0 + 128, :], in_=o_sb)

                        for qb in range(NQB):
                            q0 = qb * QB
                            for h in range(NH):
                                outT = opool.tile(
                                    [VA, QB], F32, tag="ot", name="outT")
                                for kc in range(SC):
                                    sco = spool.tile(
                                        [128, QB], F32, tag="sc", name="sco")
                                    for nn in range(QB // 512):
                                        nc.tensor.matmul(
                                            sco[:, nn * 512:(nn + 1) * 512],
                                            kT[:, h, kc * 128:(kc + 1) * 128],
                                            qT[:, h,
                                               q0 + nn * 512:q0 + (nn + 1) * 512],
                                            start=True, stop=True,
                                        )
                                    ex = expp.tile(
                                        [128, QB], BF16, tag="ex", name="ex")
                                    nc.scalar.activation(
                                        out=ex, in_=sco, func=EXP)
                                    for nn in range(QB // 512):
                                        nc.tensor.matmul(
                                            outT[:, nn * 512:(nn + 1) * 512],
                                            vaug[:, kc, h, :],
                                            ex[:, nn * 512:(nn + 1) * 512],
                                            start=(kc == 0), stop=(kc == SC - 1),
                                        )
                                nc.vector.tensor_copy(
                                    raw_m[:, h, q0:q0 + QB], outT)
                                # per-head denominator pipeline + merge: the
                                # [1,1024] denominator row is DMA-folded to
                                # [4,256] so the reciprocal runs on 4 lanes;
                                # the whole chain hides under the next head's
                                # attention block
                                nc.gpsimd.dma_start(
                                    out=dsum4[:, h, qb, :],
                                    in_=raw_m[80:81, h, q0:q0 + QB],
                                )
                                nc.vector.reciprocal_approx_fast(
                                    out=dsum4r[:, h, qb, :],
                                    in_=dsum4[:, h, qb, :])
                                nc.gpsimd.dma_start(
                                    out=rrows[0:1, 0, h, q0:q0 + QB],
                                    in_=dsum4r[:, h, qb, :],
                                )
                                bc = bcp.tile(
                                    [D, QB], BF16, tag="bcm", name="bc_m")
                                nc.gpsimd.partition_broadcast(
                                    bc, rrows[0:1, 0, h, q0:q0 + QB])
                                nc.vector.tensor_mul(
                                    osb_m[:, h, q0:q0 + QB],
                                    raw_m[0:80, h, q0:q0 + QB],
                                    bc,
                                )
                                if qb == 1 and h >= 1:
                                    # interleave part of qb0's Wo into qb1's
                                    # attention (fills ACT-paced PE slack,
                                    # keeps the PE warm); sj 3..7 are held
                                    # back to feed the PE during the final
                                    # denominator/merge chain
                                    emit_wo(h - 1, outsb_pool, opool,
                                            evac="dve")


                        # tail: reserved qb0 chunks run while qb1's merge
                        # chain completes, then qb1's chunks; evac on the
                        # now-idle Scalar engine (DVE runs the merge muls)
                        for sj in range(3, S // 128):
                            emit_wo(sj, outsb_pool, opool, evac="act")
    nc.compile()
    return nc


_NC_CACHE = {}
LAST_EXEC_NS = None
LAST_RESULT = None


def _get_nc():
    if "nc" not in _NC_CACHE:
        _NC_CACHE["nc"] = build_nc()
    return _NC_CACHE["nc"]


def make_in_maps(inputs):
    hs = np.asarray(inputs["hidden_states"], np.float32)
    au = np.asarray(inputs["au_embedding"], np.float32)
    Wq = np.asarray(inputs["Wq"], np.float32)
    Wk = np.asarray(inputs["Wk"], np.float32)
    Wv = np.asarray(inputs["Wv"], np.float32)
    Wak = np.asarray(inputs["Wak"], np.float32)
    Wav = np.asarray(inputs["Wav"], np.float32)
    null_token = np.asarray(inputs["null_token"], np.float32).reshape(1, C)
    gamma = np.asarray(inputs["gamma"], np.float32)
    Wo = np.asarray(inputs["Wo"], np.float32)

    Wq_s = Wq * SCALE
    Wav_g = Wav * gamma[None, :]

    def b16(x):
        return np.ascontiguousarray(x.astype(BF16_NP))

    in_maps = []
    for c in range(8):
        b, hg = divmod(c, 2)
        sl = slice(hg * CH, (hg + 1) * CH)
        ext = np.concatenate(
            [au[b], null_token, np.zeros((1, C), np.float32)], axis=0
        )  # [14, C]; row 13 is even-size padding
        extz = ext.copy()
        extz[NAU - 1] = 0.0
        in_maps.append({
            "hsT": b16(hs[b].T),
            "wq": b16(Wq_s[:, sl]),
            "wk": b16(Wk[:, sl]),
            "wv": b16(Wv[:, sl]),
            "wak": b16(Wak[:, sl]),
            "wav": b16(Wav_g[:, sl]),
            "wo": b16(Wo[sl, :]),
            "extT": b16(ext.T),
            "extzT": b16(extz.T),
        })
    return in_maps


def kernel(**inputs):
    global LAST_EXEC_NS, LAST_RESULT
    hs = np.asarray(inputs["hidden_states"], np.float32)
    bo = np.asarray(inputs["bo"], np.float32)
    in_maps = make_in_maps(inputs)
    nc = _get_nc()
    trace = os.environ.get("KERNEL_TRACE", "0") == "1"
    res = run_bass_kernel_spmd(nc, in_maps, list(range(8)), trace=trace)
    LAST_EXEC_NS = res.exec_time_ns
    LAST_RESULT = res
    out = np.empty((B, S, C), np.float32)
    for b in range(B):
        out[b] = res.results[2 * b]["outp"] + res.results[2 * b + 1]["outp"]
        out[b] += bo[None, :]
        out[b] += hs[b]
    return out
